# revision 4
# baseline (speedup 1.0000x reference)
"""4-bit ColBlockQuantizedLinear on 8 TRN2 cores — fp8-DoubleRow scheme.

Math: out[b,o] = scales[o] * (sum_i inp[b,i]*wq[o,i] - zeros[o]*rowsum[b]),
wq packed two nibbles per byte (even i -> low nibble, odd i -> high).

Key device trick: fp8e4m3 bit patterns 0x00..0x0F encode EXACTLY n * 2^-9
(IEEE gradual underflow: denormals + first normal binade are linear in n).
So nibble extraction is just two u32 SWAR ops per chunk on DVE —
   l-plane = q & 0x0F0F0F0F,  h-plane = (q >> 4) & 0x0F0F0F0F
— whose byte outputs ARE valid fp8 encodings of nibble/512.  No casts.
(Verified on HW: DVE runs these at 2x_2P, ~0.78us per 4-kt chunk op.)

Matmul: DoubleRow fp8 (2 fp8/cell, 2 elem/cycle ingest).  Moving operand =
[128, 2(plane), Ncols] nibble planes; stationary = [128, 2, 32] e4m3 hi/lo
split activations (cols 0:16 hi(b), 16:32 lo(b)); psum [32, Nblk] fp32.
Warm-up dummy matmuls run during the DMA fill so HAM reaches K=8/8 before
the real stream (216ns/MM warm at N=512).

DMA: q chunks alternate between the two HWDGE rings (SP + Activation) with
all chunks in flight to keep the 16 SDMA engines saturated.

Host glue (O(B*I)+O(B*O) only; all O(B*I*O) work on device): byte-layout
repack, e4m3 hi/lo split of activations, and the final combine
   out = 512*scales*(P_hi + P_lo) - scales*zeros*rowsum.

Sharding: column-parallel over out_features (1376 per core), activations
replicated; per-core [32,1376] psum images gathered and combined on host.
"""

import numpy as np
import ml_dtypes

B = 16
I = 4096
O = 11008
NCORES = 8
OS = O // NCORES          # 1376 out-features per core
HALF = I // 2             # 2048 packed byte-rows
KT = 16                   # 128-row contraction tiles
NCH = 4                   # DMA chunks
KPC = KT // NCH           # k-tiles per chunk
CHB = KPC * OS            # bytes/partition/chunk = 5504
BLKS = [(0, 512), (512, 512), (1024, 352)]
NWARM = 10                # PE warm-up dummy matmuls

FP8 = ml_dtypes.float8_e4m3fn
BF16 = ml_dtypes.bfloat16

_CACHE = {}


def _build_program():
    import concourse.bacc as bacc
    import concourse.mybir as mybir
    import concourse.tile as tile

    dt = mybir.dt
    op = mybir.AluOpType
    nc = bacc.Bacc("TRN2", target_bir_lowering=False)

    q = nc.dram_tensor("q", [128, KT * OS], dt.uint8, kind="ExternalInput")
    stat = nc.dram_tensor("stat", [128, 2 * KT * 32], dt.uint8, kind="ExternalInput")
    out = nc.dram_tensor("out", [32, OS], dt.float32, kind="ExternalOutput")

    with tile.TileContext(nc) as tc:
        with (
            tc.tile_pool(name="consts", bufs=1) as cpool,
            tc.tile_pool(name="qp", bufs=NCH) as qpool,
            tc.tile_pool(name="pl", bufs=NCH) as plpool,
            tc.tile_pool(name="ps", bufs=1, space="PSUM") as pspool,
        ):
            stat_sb = cpool.tile([128, 2 * KT * 32], dt.uint8, name="stat_sb")
            nc.sync.dma_start(stat_sb, stat[:, :])
            sf8 = stat_sb.bitcast(dt.float8e4)[:, :].rearrange(
                "p (s c) -> p s c", s=2
            )  # [128, 2, KT*32]

            psums = [
                pspool.tile([32, n], dt.float32, name=f"ps{i}")
                for i, (_, n) in enumerate(BLKS)
            ]
            # PE warm-up: garbage DoubleRow matmuls into a scratch psum bank,
            # fed entirely from stat_sb (arrives within ~1.5us).  Keeps the
            # PE busy through the HAM SHORT window so real matmuls run warm.
            warm_ps = pspool.tile([32, 512], dt.float32, name="warm_ps")
            for d in range(NWARM):
                nc.tensor.matmul(
                    warm_ps,
                    sf8[:, :, 0:32],
                    sf8[:, :, 0:512],
                    start=(d == 0),
                    stop=(d == NWARM - 1),
                    perf_mode=mybir.MatmulPerfMode.DoubleRow,
                )

            qts, planes_l = [], []
            for ch in range(NCH):
                qt = qpool.tile([128, CHB], dt.uint8, name="qt", tag="qt")
                eng = nc.sync if ch % 2 == 0 else nc.scalar
                eng.dma_start(qt, q[:, ch * CHB : (ch + 1) * CHB])
                qts.append(qt)

            for ch in range(NCH):
                qt = qts[ch]
                planes = plpool.tile([128, 2 * CHB], dt.uint8, name="pl",
                                     tag="pl")
                qu32 = qt.bitcast(dt.uint32)
                pu32 = planes.bitcast(dt.uint32)
                NU = CHB // 4
                nc.vector.tensor_scalar(
                    pu32[:, 0:NU], qu32[:, :], 0x0F0F0F0F, None, op.bitwise_and
                )
                nc.vector.tensor_scalar(
                    pu32[:, NU : 2 * NU], qu32[:, :], 4, 0x0F0F0F0F,
                    op.logical_shift_right, op.bitwise_and,
                )
                pf8 = planes.bitcast(dt.float8e4)[:, :].rearrange(
                    "p (s c) -> p s c", s=2
                )  # [128, 2, CHB]
                for k in range(KPC):
                    kt = ch * KPC + k
                    for i, (s, n) in enumerate(BLKS):
                        nc.tensor.matmul(
                            psums[i],
                            sf8[:, :, kt * 32 : kt * 32 + 32],
                            pf8[:, :, k * OS + s : k * OS + s + n],
                            start=(kt == 0),
                            stop=(kt == KT - 1),
                            perf_mode=mybir.MatmulPerfMode.DoubleRow,
                        )

            # Finalize: psum -> sbuf copies split across ACT and DVE, then one
            # output DMA per ring.
            ot = cpool.tile([32, OS], dt.float32, name="ot")
            for i, (s, n) in enumerate(BLKS):
                h = n // 2
                nc.scalar.activation(
                    ot[:, s : s + h], psums[i][:, 0:h],
                    mybir.ActivationFunctionType.Copy,
                )
                nc.vector.tensor_copy(ot[:, s + h : s + n], psums[i][:, h:n])
            nc.sync.dma_start(out[:, 0:1024], ot[:, 0:1024])
            nc.scalar.dma_start(out[:, 1024:OS], ot[:, 1024:OS])

    nc.finalize()
    return nc


def _get_program():
    if "nc" not in _CACHE:
        _CACHE["nc"] = _build_program()
    return _CACHE["nc"]


def _host_prep(inp, quant_weight, scales=None, zeros=None):
    """Layout/precision prep only (no dequant math)."""
    inp64 = np.asarray(inp, dtype=np.float64)
    a = inp64[:, 0::2].T  # [2048, B]  pairs low nibbles
    bo = inp64[:, 1::2].T  # [2048, B]  pairs high nibbles

    # stat[p, s, kt, 0:16] = e4m3 hi of act rows kt*128+p; [16:32] = e4m3 lo
    stat = np.zeros((128, 2, KT, 32), dtype=FP8)
    for s, arr in ((0, a), (1, bo)):
        arr_k = arr.reshape(KT, 128, B)  # [kt, p, b]
        hi = arr_k.astype(FP8)
        lo = (arr_k - hi.astype(np.float64)).astype(FP8)
        stat[:, s, :, 0:16] = hi.transpose(1, 0, 2)
        stat[:, s, :, 16:32] = lo.transpose(1, 0, 2)
    stat_u8 = np.ascontiguousarray(stat).reshape(128, 2 * KT * 32).view(np.uint8)

    qw = np.asarray(quant_weight)
    in_maps = []
    for c in range(NCORES):
        qc = qw[c * OS : (c + 1) * OS].astype(np.uint8).T  # [2048, OS] (j, o)
        q_dev = np.ascontiguousarray(
            qc.reshape(KT, 128, OS).transpose(1, 0, 2)
        ).reshape(128, KT * OS)
        in_maps.append({"q": q_dev, "stat": stat_u8})
    return in_maps


def kernel(inp, quant_weight, scales, zeros):
    from concourse.bass_utils import run_bass_kernel_spmd

    nc = _get_program()
    in_maps = _host_prep(inp, quant_weight)
    res = run_bass_kernel_spmd(nc, in_maps, core_ids=list(range(NCORES)))

    inp64 = np.asarray(inp, dtype=np.float64)
    rowsum = inp64.sum(axis=1)  # [B]
    sc = np.asarray(scales, dtype=np.float64).reshape(-1)
    zr = np.asarray(zeros, dtype=np.float64).reshape(-1)
    outs = []
    for c in range(NCORES):
        P = np.asarray(res.results[c]["out"], dtype=np.float64)  # [32, OS]
        s_c = sc[c * OS : (c + 1) * OS]
        z_c = zr[c * OS : (c + 1) * OS]
        S = (P[:B] + P[B : 2 * B]) * 512.0
        outs.append(S * s_c[None, :] - rowsum[:, None] * (s_c * z_c)[None, :])
    out = np.concatenate(outs, axis=1).astype(np.float32)
    return np.ascontiguousarray(out)


# revision 6
# speedup vs baseline: 1.1120x; 1.1120x over previous
"""4-bit ColBlockQuantizedLinear on 8 TRN2 cores — fp8-DoubleRow scheme.

Math: out[b,o] = scales[o] * (sum_i inp[b,i]*wq[o,i] - zeros[o]*rowsum[b]),
wq packed two nibbles per byte (even i -> low nibble, odd i -> high).

Key device trick: fp8e4m3 bit patterns 0x00..0x0F encode EXACTLY n * 2^-9
(IEEE gradual underflow: denormals + first normal binade are linear in n).
So nibble extraction is just two u32 SWAR ops per chunk on DVE —
   l-plane = q & 0x0F0F0F0F,  h-plane = (q >> 4) & 0x0F0F0F0F
— whose byte outputs ARE valid fp8 encodings of nibble/512.  No casts.
(Verified on HW: DVE runs these at 2x_2P, ~0.78us per 4-kt chunk op.)

Matmul: DoubleRow fp8 (2 fp8/cell, 2 elem/cycle ingest).  Moving operand =
[128, 2(plane), Ncols] nibble planes; stationary = [128, 2, 32] e4m3 hi/lo
split activations (cols 0:16 hi(b), 16:32 lo(b)); psum [32, Nblk] fp32.
Warm-up dummy matmuls run during the DMA fill so HAM reaches K=8/8 before
the real stream (216ns/MM warm at N=512).

DMA: q chunks alternate between the two HWDGE rings (SP + Activation) with
all chunks in flight to keep the 16 SDMA engines saturated.

Host glue (O(B*I)+O(B*O) only; all O(B*I*O) work on device): byte-layout
repack, e4m3 hi/lo split of activations, and the final combine
   out = 512*scales*(P_hi + P_lo) - scales*zeros*rowsum.

Sharding: column-parallel over out_features (1376 per core), activations
replicated; per-core [32,1376] psum images gathered and combined on host.
"""

import numpy as np
import ml_dtypes

B = 16
I = 4096
O = 11008
NCORES = 8
OS = O // NCORES          # 1376 out-features per core
HALF = I // 2             # 2048 packed byte-rows
KT = 16                   # 128-row contraction tiles
# Staggered chunk sizes (in k-tiles), alternating between the two HWDGE
# rings (SP, ACT).  Small chunks first so kt0 lands early while later,
# larger chunks keep all 16 SDMA engines saturated (per-ring FIFO).
CH_KT = [1, 1, 2, 2, 2, 2, 3, 3]
BLKS = [(0, 512), (512, 512), (1024, 352)]
NWARM = 8                 # PE warm-up dummy matmuls

FP8 = ml_dtypes.float8_e4m3fn
BF16 = ml_dtypes.bfloat16

_CACHE = {}


def _build_program():
    import concourse.bacc as bacc
    import concourse.mybir as mybir
    import concourse.tile as tile

    dt = mybir.dt
    op = mybir.AluOpType
    nc = bacc.Bacc("TRN2", target_bir_lowering=False)

    q = nc.dram_tensor("q", [128, KT * OS], dt.uint8, kind="ExternalInput")
    stat = nc.dram_tensor("stat", [128, 2 * KT * 32], dt.uint8, kind="ExternalInput")
    out = nc.dram_tensor("out", [32, OS], dt.float32, kind="ExternalOutput")

    with tile.TileContext(nc) as tc:
        with (
            tc.tile_pool(name="consts", bufs=1) as cpool,
            tc.tile_pool(name="qp", bufs=1) as qpool,
            tc.tile_pool(name="pl", bufs=1) as plpool,
            tc.tile_pool(name="ps", bufs=1, space="PSUM") as pspool,
        ):
            # PE warm-up: garbage DoubleRow matmuls on a memset scratch tile
            # (0x38 = e4m3 1.0) into a scratch psum bank, starting right after
            # the NEFF preamble so HAM reaches K=8/8 before the real stream.
            warm_sb = cpool.tile([128, 1024], dt.uint8, name="warm_sb")
            nc.gpsimd.memset(warm_sb, 0x38)
            wf8 = warm_sb.bitcast(dt.float8e4)[:, :].rearrange(
                "p (s c) -> p s c", s=2
            )
            warm_ps = pspool.tile([32, 512], dt.float32, name="warm_ps")
            for d in range(NWARM):
                nc.tensor.matmul(
                    warm_ps,
                    wf8[:, :, 0:32],
                    wf8[:, :, 0:512],
                    start=(d == 0),
                    stop=(d == NWARM - 1),
                    perf_mode=mybir.MatmulPerfMode.DoubleRow,
                )

            stat_sb = cpool.tile([128, 2 * KT * 32], dt.uint8, name="stat_sb")
            nc.scalar.dma_start(stat_sb, stat[:, :])
            sf8 = stat_sb.bitcast(dt.float8e4)[:, :].rearrange(
                "p (s c) -> p s c", s=2
            )  # [128, 2, KT*32]

            psums = [
                pspool.tile([32, n], dt.float32, name=f"ps{i}")
                for i, (_, n) in enumerate(BLKS)
            ]

            # Chunked q DMAs, alternating rings (even -> SP, odd -> ACT).
            qts = []
            kt0 = 0
            for ch, nkt in enumerate(CH_KT):
                cb = nkt * OS
                qt = qpool.tile([128, cb], dt.uint8, name=f"qt{ch}",
                                tag=f"qt{ch}")
                eng = nc.sync if ch % 2 == 0 else nc.scalar
                eng.dma_start(qt, q[:, kt0 * OS : kt0 * OS + cb])
                qts.append((qt, kt0, nkt))
                kt0 += nkt

            for ch, (qt, kt0, nkt) in enumerate(qts):
                cb = nkt * OS
                planes = plpool.tile([128, 2 * cb], dt.uint8, name=f"pl{ch}",
                                     tag=f"pl{ch}")
                qu32 = qt.bitcast(dt.uint32)
                pu32 = planes.bitcast(dt.uint32)
                NU = cb // 4
                nc.vector.tensor_scalar(
                    pu32[:, 0:NU], qu32[:, :], 0x0F0F0F0F, None, op.bitwise_and
                )
                nc.vector.tensor_scalar(
                    pu32[:, NU : 2 * NU], qu32[:, :], 4, 0x0F0F0F0F,
                    op.logical_shift_right, op.bitwise_and,
                )
                pf8 = planes.bitcast(dt.float8e4)[:, :].rearrange(
                    "p (s c) -> p s c", s=2
                )  # [128, 2, cb]
                for k in range(nkt):
                    kt = kt0 + k
                    for i, (s, n) in enumerate(BLKS):
                        nc.tensor.matmul(
                            psums[i],
                            sf8[:, :, kt * 32 : kt * 32 + 32],
                            pf8[:, :, k * OS + s : k * OS + s + n],
                            start=(kt == 0),
                            stop=(kt == KT - 1),
                            perf_mode=mybir.MatmulPerfMode.DoubleRow,
                        )

            # Finalize: psum -> sbuf copies split across ACT and DVE, then one
            # output DMA per ring.
            ot = cpool.tile([32, OS], dt.float32, name="ot")
            for i, (s, n) in enumerate(BLKS):
                h = n // 2
                nc.scalar.activation(
                    ot[:, s : s + h], psums[i][:, 0:h],
                    mybir.ActivationFunctionType.Copy,
                )
                nc.vector.tensor_copy(ot[:, s + h : s + n], psums[i][:, h:n])
            nc.sync.dma_start(out[:, 0:1024], ot[:, 0:1024])
            nc.scalar.dma_start(out[:, 1024:OS], ot[:, 1024:OS])

    nc.finalize()
    return nc


def _get_program():
    if "nc" not in _CACHE:
        _CACHE["nc"] = _build_program()
    return _CACHE["nc"]


def _host_prep(inp, quant_weight, scales=None, zeros=None):
    """Layout/precision prep only (no dequant math)."""
    inp64 = np.asarray(inp, dtype=np.float64)
    a = inp64[:, 0::2].T  # [2048, B]  pairs low nibbles
    bo = inp64[:, 1::2].T  # [2048, B]  pairs high nibbles

    # stat[p, s, kt, 0:16] = e4m3 hi of act rows kt*128+p; [16:32] = e4m3 lo
    stat = np.zeros((128, 2, KT, 32), dtype=FP8)
    for s, arr in ((0, a), (1, bo)):
        arr_k = arr.reshape(KT, 128, B)  # [kt, p, b]
        hi = arr_k.astype(FP8)
        lo = (arr_k - hi.astype(np.float64)).astype(FP8)
        stat[:, s, :, 0:16] = hi.transpose(1, 0, 2)
        stat[:, s, :, 16:32] = lo.transpose(1, 0, 2)
    stat_u8 = np.ascontiguousarray(stat).reshape(128, 2 * KT * 32).view(np.uint8)

    qw = np.asarray(quant_weight)
    in_maps = []
    for c in range(NCORES):
        qc = qw[c * OS : (c + 1) * OS].astype(np.uint8).T  # [2048, OS] (j, o)
        q_dev = np.ascontiguousarray(
            qc.reshape(KT, 128, OS).transpose(1, 0, 2)
        ).reshape(128, KT * OS)
        in_maps.append({"q": q_dev, "stat": stat_u8})
    return in_maps


def kernel(inp, quant_weight, scales, zeros):
    from concourse.bass_utils import run_bass_kernel_spmd

    nc = _get_program()
    in_maps = _host_prep(inp, quant_weight)
    res = run_bass_kernel_spmd(nc, in_maps, core_ids=list(range(NCORES)))

    inp64 = np.asarray(inp, dtype=np.float64)
    rowsum = inp64.sum(axis=1)  # [B]
    sc = np.asarray(scales, dtype=np.float64).reshape(-1)
    zr = np.asarray(zeros, dtype=np.float64).reshape(-1)
    outs = []
    for c in range(NCORES):
        P = np.asarray(res.results[c]["out"], dtype=np.float64)  # [32, OS]
        s_c = sc[c * OS : (c + 1) * OS]
        z_c = zr[c * OS : (c + 1) * OS]
        S = (P[:B] + P[B : 2 * B]) * 512.0
        outs.append(S * s_c[None, :] - rowsum[:, None] * (s_c * z_c)[None, :])
    out = np.concatenate(outs, axis=1).astype(np.float32)
    return np.ascontiguousarray(out)
